# revision 4
# baseline (speedup 1.0000x reference)
"""GTrXL layer (TransformerXL attention + GRU gating) on 8 TRN2 NeuronCores.

Sharding: pure data-parallel over batch (BS=8 -> 1 batch element per core).
No collectives. Per-core Bass/Tile kernel computes the full layer for its
batch element.

Layout convention on-chip: activations are kept TRANSPOSED [feature, token]
(feature on partitions, 128-chunks) so that weight matrices in natural [K, N]
layout serve as the stationary matmul operand and matmul outputs land
transposed again:  outT[n, t] = sum_k W[k, n] * xT[k, t].

Matmul compute in bf16 (f32 accumulate in PSUM); LN/softmax/GRU elementwise
math in f32.

Relative-shift: pos scores P[i, relk] are written per 128-query-chunk to a
DRAM scratch of row stride 1536 whose tail 512 columns are pre-filled with
-1e30; the shifted read  shifted[i, j] = P[i, 511 + j - i]  is a single
strided DMA (row step 1535), and the pad lands exactly on the masked region
j > i + 512, so masking comes for free.
"""

import sys

if '/opt/trn_rl_repo' not in sys.path:
    sys.path.insert(0, '/opt/trn_rl_repo')

import numpy as np
import ml_dtypes

import concourse.bass as bass
import concourse.tile as tile
from concourse import bacc, mybir
from concourse.bass_utils import run_bass_kernel_spmd
from concourse.masks import make_identity

BF16 = mybir.dt.bfloat16
F32 = mybir.dt.float32

HEAD_NUM, HEAD_DIM = 16, 64
D, HID = 1024, 4096
CUR, PREV, BS = 512, 512, 8
FULL = CUR + PREV
EPS = 1e-5
SCALE = 1.0 / (HEAD_DIM ** 0.5)
P = 128
DC = D // P          # 8 feature chunks
HC = HID // P        # 32 hidden chunks
TCF = FULL // P      # 8 full-token chunks
TCC = CUR // P       # 4 query-token chunks
NEG = -1.0e30

AluOp = mybir.AluOpType
Act = mybir.ActivationFunctionType


def _dram_in(dram, name, shape, dtype):
    return dram.tile(list(shape), dtype, kind="ExternalInput", name=name,
                     uniquify=False)


def _mm_chain(nc, psum, lhsT_tiles, rhs_tiles):
    n = len(lhsT_tiles)
    for i in range(n):
        nc.tensor.matmul(psum, lhsT=lhsT_tiles[i], rhs=rhs_tiles[i],
                         start=(i == 0), stop=(i == n - 1))


def _build():
    nc = bacc.Bacc("TRN2", target_bir_lowering=False)
    with tile.TileContext(nc) as tc:
        _emit(nc, tc)
    nc.compile()
    return nc


def _emit(nc, tc):
    from contextlib import ExitStack

    with ExitStack() as root:
        dram = root.enter_context(tc.tile_pool(name="io", bufs=1, space="DRAM"))

        # ---------------- DRAM I/O ----------------
        x_full = _dram_in(dram, "x_full", (FULL, D), F32)
        inpT_d = _dram_in(dram, "inpT", (D, CUR), F32)
        posT_d = _dram_in(dram, "posT", (D, FULL), BF16)
        u_d = _dram_in(dram, "u_t", (P, DC), F32)
        v_d = _dram_in(dram, "v_t", (P, DC), F32)
        ln1g_d = _dram_in(dram, "ln1_g", (D,), F32)
        ln1b_d = _dram_in(dram, "ln1_b", (D,), F32)
        ln2g_d = _dram_in(dram, "ln2_g_t", (P, DC), F32)
        ln2b_d = _dram_in(dram, "ln2_b_t", (P, DC), F32)
        bkvK_d = _dram_in(dram, "bkvK_t", (P, DC), F32)
        bkvV_d = _dram_in(dram, "bkvV_row", (1, D), BF16)
        bq_d = _dram_in(dram, "bq_t", (P, DC), F32)
        bpos_d = _dram_in(dram, "bpos_t", (P, DC), F32)
        bproj_d = _dram_in(dram, "bproj_t", (P, DC), F32)
        b1_d = _dram_in(dram, "b1_t", (P, HC), F32)
        b2_d = _dram_in(dram, "b2_t", (P, DC), F32)
        nbg1_d = _dram_in(dram, "nbg1_t", (P, DC), F32)
        nbg2_d = _dram_in(dram, "nbg2_t", (P, DC), F32)

        wkv_d = _dram_in(dram, "Wkv", (D, 2 * D), BF16)
        wq_d = _dram_in(dram, "Wq", (D, D), BF16)
        wpos_d = _dram_in(dram, "Wpos", (D, D), BF16)
        wproj_d = _dram_in(dram, "Wproj", (D, D), BF16)
        gw_d = {}
        for g in (1, 2):
            for m in ("Wr", "Ur", "Wz", "Uz", "Wg", "Ug"):
                gw_d[(g, m)] = _dram_in(dram, f"g{g}_{m}", (D, D), BF16)
        w1_d = _dram_in(dram, "mlp_W1", (D, HID), BF16)
        w2_d = _dram_in(dram, "mlp_W2", (HID, D), BF16)

        out_d = dram.tile([CUR, D], F32, kind="ExternalOutput", name="out",
                          uniquify=False)

        n_scr = 6
        scr = [dram.tile([P, 1536], BF16, name=f"scr{s}") for s in range(n_scr)]

        # ---------------- constants ----------------
        const = root.enter_context(tc.tile_pool(name="const", bufs=1))
        ident_f = const.tile([P, P], F32)
        make_identity(nc, ident_f)
        ident_b = const.tile([P, P], BF16)
        make_identity(nc, ident_b)
        ones_row = const.tile([1, P], BF16)
        nc.vector.memset(ones_row, 1.0)
        ones_red = const.tile([P, 1], BF16)
        nc.vector.memset(ones_red, 1.0)
        eps_t = const.tile([P, 1], F32)
        nc.vector.memset(eps_t, EPS)

        def cload(name, dref, shape, dtype=F32):
            t = const.tile(list(shape), dtype, name=name)
            nc.sync.dma_start(out=t, in_=dref[:])
            return t

        u_sb = cload("u_sb", u_d, (P, DC))
        v_sb = cload("v_sb", v_d, (P, DC))
        ln2g_sb = cload("ln2g_sb", ln2g_d, (P, DC))
        ln2b_sb = cload("ln2b_sb", ln2b_d, (P, DC))
        bkvK_sb = cload("bkvK_sb", bkvK_d, (P, DC))
        bkvV_sb = cload("bkvV_sb", bkvV_d, (1, D), BF16)
        bq_sb = cload("bq_sb", bq_d, (P, DC))
        bpos_sb = cload("bpos_sb", bpos_d, (P, DC))
        bproj_sb = cload("bproj_sb", bproj_d, (P, DC))
        b1_sb = cload("b1_sb", b1_d, (P, HC))
        b2_sb = cload("b2_sb", b2_d, (P, DC))
        nbg1_sb = cload("nbg1_sb", nbg1_d, (P, DC))
        nbg2_sb = cload("nbg2_sb", nbg2_d, (P, DC))

        padw = const.tile([P, 512], BF16)
        nc.vector.memset(padw, NEG)
        for s in range(n_scr):
            nc.sync.dma_start(out=scr[s][:, 1024:1536], in_=padw)

        # shared psum pools (4 + 2 + 2 = 8 banks)
        psum = root.enter_context(tc.tile_pool(name="psum", bufs=4, space="PSUM"))
        psum_t = root.enter_context(tc.tile_pool(name="psum_t", bufs=2, space="PSUM"))
        psum_s = root.enter_context(tc.tile_pool(name="psum_s", bufs=2, space="PSUM"))

        def PS():
            return psum.tile([P, 512], F32, name="ps", tag="ps")

        def PT(dtype):
            return psum_t.tile([P, P], dtype, name="pt", tag="pt")

        def SM():
            return psum_s.tile([1, 512], F32, name="sm", tag="sm")

        # lifetime-managed activations (two-sided stack allocator:
        # frees must be LIFO per side, so lifetimes are laid out on
        # left/right stacks to nest properly)
        def mk(name, shape, dtype, side):
            t, fr = tc.tile(list(shape), dtype, name=name, side=side)
            return t, fr

        x1T, fr_x1T = mk("x1T", (P, DC, FULL), BF16, "left")

        # ================= Phase 1: LN1 + transpose =================
        with ExitStack() as ph:
            ln1c = ph.enter_context(tc.tile_pool(name="ln1c", bufs=1, side="left"))
            ln1g_sb = ln1c.tile([P, D], F32)
            nc.sync.dma_start(out=ln1g_sb, in_=bass.AP(
                tensor=ln1g_d.tensor, offset=ln1g_d.offset, ap=[[0, P], [1, D]]))
            ln1b_sb = ln1c.tile([P, D], F32)
            nc.sync.dma_start(out=ln1b_sb, in_=bass.AP(
                tensor=ln1b_d.tensor, offset=ln1b_d.offset, ap=[[0, P], [1, D]]))

            xw = ph.enter_context(tc.tile_pool(name="xw", bufs=3, side="left"))
            st = ph.enter_context(tc.tile_pool(name="st", bufs=3, side="left"))
            x_t = x_full[:].rearrange("(tc p) d -> p tc d", p=P)
            for tcx in range(TCF):
                xt = xw.tile([P, D], F32, name="xt")
                nc.sync.dma_start(out=xt, in_=x_t[:, tcx, :])
                stats = st.tile([P, 2, 6], F32, name="stats")
                nc.vector.bn_stats(out=stats[:, 0, :], in_=xt[:, 0:512])
                nc.vector.bn_stats(out=stats[:, 1, :], in_=xt[:, 512:1024])
                mv = st.tile([P, 2], F32, name="mv")
                nc.vector.bn_aggr(out=mv, in_=stats)
                sd = st.tile([P, 1], F32, name="sd")
                nc.scalar.activation(out=sd, in_=mv[:, 1:2], func=Act.Sqrt,
                                     bias=eps_t)
                rstd = st.tile([P, 1], F32, name="rstd")
                nc.vector.reciprocal(out=rstd, in_=sd)
                xn = xw.tile([P, D], F32, name="xn")
                nc.vector.tensor_scalar(out=xn, in0=xt, scalar1=mv[:, 0:1],
                                        scalar2=rstd, op0=AluOp.subtract,
                                        op1=AluOp.mult)
                x1n = xw.tile([P, D], F32, name="x1n")
                nc.vector.scalar_tensor_tensor(out=x1n, in0=xn, scalar=1.0,
                                               in1=ln1g_sb, op0=AluOp.mult,
                                               op1=AluOp.mult)
                nc.vector.tensor_add(x1n, x1n, ln1b_sb)
                for dc in range(DC):
                    pt = PT(F32)
                    nc.tensor.transpose(pt, x1n[:, dc * P:(dc + 1) * P], ident_f)
                    nc.scalar.copy(x1T[:, dc, tcx * P:(tcx + 1) * P], pt)

        # ================= Phase 2: KT, V, qT, rT =================
        kT, fr_kT = mk("kT", (P, DC, FULL), BF16, "right")
        v_nat, fr_v = mk("v_nat", (P, TCF, D), BF16, "right")
        rT, fr_rT = mk("rT", (P, DC, FULL), BF16, "right")
        quT, fr_quT = mk("quT", (P, DC, CUR), BF16, "right")
        qvT, fr_qvT = mk("qvT", (P, DC, CUR), BF16, "right")

        with ExitStack() as ph:
            wkvp = ph.enter_context(tc.tile_pool(name="wkvp", bufs=1, side="right"))
            wkv = wkvp.tile([P, DC, 2 * D], BF16)
            nc.sync.dma_start(out=wkv, in_=wkv_d[:].rearrange("(kc p) n -> p kc n", p=P))
            for n in range(DC):
                for th in range(2):
                    ps = PS()
                    _mm_chain(nc, ps,
                              [wkv[:, k, n * P:(n + 1) * P] for k in range(DC)],
                              [x1T[:, k, th * 512:(th + 1) * 512] for k in range(DC)])
                    nc.scalar.activation(out=kT[:, n, th * 512:(th + 1) * 512],
                                         in_=ps, func=Act.Identity,
                                         bias=bkvK_sb[:, n:n + 1])
            for t in range(TCF):
                for nh in range(2):
                    ps = PS()
                    for k in range(DC):
                        nc.tensor.matmul(ps, lhsT=x1T[:, k, t * P:(t + 1) * P],
                                         rhs=wkv[:, k, D + nh * 512:D + (nh + 1) * 512],
                                         start=(k == 0), stop=False)
                    nc.tensor.matmul(ps, lhsT=ones_row,
                                     rhs=bkvV_sb[:, nh * 512:(nh + 1) * 512],
                                     start=False, stop=True)
                    nc.scalar.copy(v_nat[:, t, nh * 512:(nh + 1) * 512], ps)
        with ExitStack() as ph:
            wqp = ph.enter_context(tc.tile_pool(name="wqp", bufs=1, side="right"))
            wq = wqp.tile([P, DC, D], BF16)
            nc.sync.dma_start(out=wq, in_=wq_d[:].rearrange("(kc p) n -> p kc n", p=P))
            qw = ph.enter_context(tc.tile_pool(name="qw", bufs=3, side="right"))
            for n in range(DC):
                ps = PS()
                _mm_chain(nc, ps,
                          [wq[:, k, n * P:(n + 1) * P] for k in range(DC)],
                          [x1T[:, k, CUR:FULL] for k in range(DC)])
                qn = qw.tile([P, 512], F32, name="qn")
                nc.scalar.activation(out=qn, in_=ps, func=Act.Identity,
                                     bias=bq_sb[:, n:n + 1])
                nc.vector.tensor_scalar_add(quT[:, n, :], qn, u_sb[:, n:n + 1])
                nc.vector.tensor_scalar_add(qvT[:, n, :], qn, v_sb[:, n:n + 1])
        with ExitStack() as ph:
            wpp = ph.enter_context(tc.tile_pool(name="wpp", bufs=1, side="right"))
            wpos = wpp.tile([P, DC, D], BF16)
            nc.sync.dma_start(out=wpos, in_=wpos_d[:].rearrange("(kc p) n -> p kc n", p=P))
            posT_sb = wpp.tile([P, DC, FULL], BF16)
            nc.sync.dma_start(out=posT_sb, in_=posT_d[:].rearrange("(kc p) f -> p kc f", p=P))
            for n in range(DC):
                for fh in range(2):
                    ps = PS()
                    _mm_chain(nc, ps,
                              [wpos[:, k, n * P:(n + 1) * P] for k in range(DC)],
                              [posT_sb[:, k, fh * 512:(fh + 1) * 512] for k in range(DC)])
                    nc.scalar.activation(out=rT[:, n, fh * 512:(fh + 1) * 512],
                                         in_=ps, func=Act.Identity,
                                         bias=bpos_sb[:, n:n + 1])
        fr_x1T()

        # ================= Phase 3: attention =================
        avT, fr_avT = mk("avT", (P, DC, CUR), BF16, "left")
        with ExitStack() as ph:
            aw = ph.enter_context(tc.tile_pool(name="aw", bufs=3, side="left"))
            atw = ph.enter_context(tc.tile_pool(name="atw", bufs=2, side="left"))
            rw = ph.enter_context(tc.tile_pool(name="rw", bufs=2, side="left"))
            scnt = 0
            for h in range(HEAD_NUM):
                ch, rb = h // 2, (h % 2) * HEAD_DIM
                quh = quT[rb:rb + HEAD_DIM, ch, :]
                qvh = qvT[rb:rb + HEAD_DIM, ch, :]
                kh = kT[rb:rb + HEAD_DIM, ch, :]
                rh = rT[rb:rb + HEAD_DIM, ch, :]
                attnT = atw.tile([P, TCF, 512], BF16, name="attnT")
                for ic in range(TCC):
                    s_t = scr[scnt % n_scr]
                    scnt += 1
                    pb = aw.tile([P, FULL], BF16, name="pb")
                    for jh in range(2):
                        pp = PS()
                        nc.tensor.matmul(pp, lhsT=qvh[:, ic * P:(ic + 1) * P],
                                         rhs=rh[:, jh * 512:(jh + 1) * 512],
                                         start=True, stop=True)
                        nc.scalar.copy(pb[:, jh * 512:(jh + 1) * 512], pp)
                    nc.sync.dma_start(out=s_t[:, 0:1024], in_=pb)
                    shp = aw.tile([P, FULL], BF16, name="shp")
                    shift_ap = bass.AP(tensor=s_t.tensor, offset=s_t.offset + 511,
                                       ap=[[1535, P], [1, FULL]])
                    nc.sync.dma_start(out=shp, in_=shift_ap)
                    es = aw.tile([P, FULL], BF16, name="es")
                    for jh in range(2):
                        cp = PS()
                        nc.tensor.matmul(cp, lhsT=quh[:, ic * P:(ic + 1) * P],
                                         rhs=kh[:, jh * 512:(jh + 1) * 512],
                                         start=True, stop=True)
                        sm = aw.tile([P, 512], F32, name="smadd")
                        nc.vector.tensor_add(sm, cp, shp[:, jh * 512:(jh + 1) * 512])
                        nc.scalar.activation(out=es[:, jh * 512:(jh + 1) * 512],
                                             in_=sm, func=Act.Exp, scale=SCALE)
                    for jc in range(TCF):
                        pt = PT(BF16)
                        nc.tensor.transpose(pt, es[:, jc * P:(jc + 1) * P], ident_b)
                        nc.scalar.copy(attnT[:, jc, ic * P:(ic + 1) * P], pt)
                dn = SM()
                _mm_chain(nc, dn, [ones_red] * TCF,
                          [attnT[:, jc, :] for jc in range(TCF)])
                recip = rw.tile([1, 512], F32, name="recip")
                nc.vector.reciprocal(out=recip, in_=dn)
                recipB = rw.tile([HEAD_DIM, 512], F32, name="recipB")
                nc.gpsimd.partition_broadcast(recipB, recip)
                av = PS()
                for jc in range(TCF):
                    nc.tensor.matmul(av[0:HEAD_DIM, :],
                                     lhsT=v_nat[:, jc, h * HEAD_DIM:(h + 1) * HEAD_DIM],
                                     rhs=attnT[:, jc, :],
                                     start=(jc == 0), stop=(jc == TCF - 1))
                nc.vector.tensor_mul(avT[rb:rb + HEAD_DIM, ch, :],
                                     av[0:HEAD_DIM, :], recipB)
        fr_qvT(); fr_quT(); fr_rT(); fr_v(); fr_kT()

        # ================= Phase 4: proj + GRU1 =================
        a1T, fr_a1T = mk("a1T", (P, DC, CUR), BF16, "right")
        with ExitStack() as ph:
            wpr = ph.enter_context(tc.tile_pool(name="wpr", bufs=1, side="left"))
            wproj = wpr.tile([P, DC, D], BF16)
            nc.sync.dma_start(out=wproj, in_=wproj_d[:].rearrange("(kc p) n -> p kc n", p=P))
            for n in range(DC):
                ps = PS()
                _mm_chain(nc, ps,
                          [wproj[:, k, n * P:(n + 1) * P] for k in range(DC)],
                          [avT[:, k, :] for k in range(DC)])
                nc.scalar.activation(out=a1T[:, n, :], in_=ps, func=Act.Relu,
                                     bias=bproj_sb[:, n:n + 1])
        fr_avT()

        o1T_f, fr_o1f = mk("o1T_f", (P, DC, CUR), F32, "left")
        o1T_b, fr_o1b = mk("o1T_b", (P, DC, CUR), BF16, "left")
        inpT_f, fr_inpf = mk("inpT_f", (P, DC, CUR), F32, "left")
        inpT_b, fr_inpb = mk("inpT_b", (P, DC, CUR), BF16, "left")
        nc.sync.dma_start(out=inpT_f, in_=inpT_d[:].rearrange("(kc p) t -> p kc t", p=P))
        nc.vector.tensor_copy(inpT_b, inpT_f)
        with ExitStack() as ph:
            _gru(nc, tc, ph, PS, gw_d, 1, a1T, inpT_b, inpT_f, nbg1_sb,
                 o1T_f, o1T_b)
        fr_inpb(); fr_inpf(); fr_a1T()

        # ================= Phase 5: LN2 =================
        x2T, fr_x2T = mk("x2T", (P, DC, CUR), BF16, "right")
        with ExitStack() as ph:
            lw = ph.enter_context(tc.tile_pool(name="lw", bufs=2, side="left"))
            sqp = ph.enter_context(tc.tile_pool(name="sqp", bufs=1, side="left"))
            sq = sqp.tile([P, DC, 512], BF16, name="sq")
            for n in range(DC):
                nc.vector.tensor_mul(sq[:, n, :], o1T_f[:, n, :], o1T_f[:, n, :])
            s1 = SM()
            _mm_chain(nc, s1, [ones_red] * DC, [o1T_b[:, n, :] for n in range(DC)])
            mean = lw.tile([1, 512], F32, name="mean")
            nc.vector.tensor_scalar_mul(mean, s1, 1.0 / D)
            s2 = SM()
            _mm_chain(nc, s2, [ones_red] * DC, [sq[:, n, :] for n in range(DC)])
            m2m = lw.tile([1, 512], F32, name="m2m")
            nc.vector.tensor_scalar_mul(m2m, s2, 1.0 / D)
            var = lw.tile([1, 512], F32, name="var")
            nc.vector.scalar_tensor_tensor(out=var, in0=mean, scalar=1.0,
                                           in1=mean, op0=AluOp.mult,
                                           op1=AluOp.mult)
            nc.vector.tensor_sub(var, m2m, var)
            sd = lw.tile([1, 512], F32, name="sd2")
            nc.scalar.activation(out=sd, in_=var, func=Act.Sqrt,
                                 bias=eps_t[0:1, :])
            rstd = lw.tile([1, 512], F32, name="rstd2")
            nc.vector.reciprocal(out=rstd, in_=sd)
            meanB = lw.tile([P, 512], F32, name="meanB")
            nc.gpsimd.partition_broadcast(meanB, mean)
            rstdB = lw.tile([P, 512], F32, name="rstdB")
            nc.gpsimd.partition_broadcast(rstdB, rstd)
            for n in range(DC):
                t1 = lw.tile([P, 512], F32, name="t1")
                nc.vector.tensor_sub(t1, o1T_f[:, n, :], meanB)
                nc.vector.tensor_mul(t1, t1, rstdB)
                nc.vector.tensor_scalar(out=x2T[:, n, :], in0=t1,
                                        scalar1=ln2g_sb[:, n:n + 1],
                                        scalar2=ln2b_sb[:, n:n + 1],
                                        op0=AluOp.mult, op1=AluOp.add)

        # ================= Phase 6: MLP =================
        with ExitStack() as ph6:
            m1w = ph6.enter_context(tc.tile_pool(name="m1w", bufs=1, side="right"))
            m1T = m1w.tile([P, HC, 512], BF16)
            with ExitStack() as ph:
                w1p = ph.enter_context(tc.tile_pool(name="w1p", bufs=1, side="right"))
                w1 = w1p.tile([P, DC, HID], BF16)
                nc.sync.dma_start(out=w1, in_=w1_d[:].rearrange("(kc p) n -> p kc n", p=P))
                for n in range(HC):
                    ps = PS()
                    _mm_chain(nc, ps,
                              [w1[:, k, n * P:(n + 1) * P] for k in range(DC)],
                              [x2T[:, k, :] for k in range(DC)])
                    nc.scalar.activation(out=m1T[:, n, :], in_=ps, func=Act.Relu,
                                         bias=b1_sb[:, n:n + 1])
            m2T, fr_m2T = mk("m2T", (P, DC, CUR), BF16, "left")
            ph = ph6
            w2p = ph.enter_context(tc.tile_pool(name="w2p", bufs=2, side="left"))
            acc = ph.enter_context(tc.tile_pool(name="m2acc", bufs=1, side="left"))
            m2a = acc.tile([P, DC, 512], F32)
            w2r = w2_d[:].rearrange("(kg kc p) n -> p kg kc n", p=P, kg=4)
            for kg in range(4):
                w2t = w2p.tile([P, DC, D], BF16, name="w2t")
                nc.sync.dma_start(out=w2t, in_=w2r[:, kg, :, :])
                for n in range(DC):
                    ps = PS()
                    _mm_chain(nc, ps,
                              [w2t[:, k, n * P:(n + 1) * P] for k in range(DC)],
                              [m1T[:, kg * DC + k, :] for k in range(DC)])
                    if kg == 0:
                        nc.vector.tensor_copy(m2a[:, n, :], ps)
                    else:
                        nc.vector.tensor_add(m2a[:, n, :], m2a[:, n, :], ps)
            for n in range(DC):
                nc.scalar.activation(out=m2T[:, n, :], in_=m2a[:, n, :],
                                     func=Act.Relu, bias=b2_sb[:, n:n + 1])
        fr_x2T()

        # ================= Phase 7: GRU2 =================
        o2T_f, fr_o2 = mk("o2T_f", (P, DC, CUR), F32, "right")
        with ExitStack() as ph:
            _gru(nc, tc, ph, PS, gw_d, 2, m2T, o1T_b, o1T_f, nbg2_sb,
                 o2T_f, None)
        fr_m2T(); fr_o1b(); fr_o1f()

        # ================= Phase 8: transpose out =================
        with ExitStack() as ph:
            ow = ph.enter_context(tc.tile_pool(name="ow", bufs=2, side="left"))
            for t in range(TCC):
                on = ow.tile([P, D], F32, name="on")
                for n in range(DC):
                    pt = PT(F32)
                    nc.tensor.transpose(pt, o2T_f[:, n, t * P:(t + 1) * P], ident_f)
                    nc.scalar.copy(on[:, n * P:(n + 1) * P], pt)
                nc.sync.dma_start(out=out_d[t * P:(t + 1) * P, :], in_=on)
        fr_o2()


def _gru(nc, tc, ph, PS, gw_d, g, yT, xT_b, xT_f, nbg_sb, oT_f, oT_b):
    gwp = ph.enter_context(tc.tile_pool(name=f"gw{g}", bufs=3, side="left"))
    gtmp = ph.enter_context(tc.tile_pool(name=f"gt{g}", bufs=2, side="left"))
    gper = ph.enter_context(tc.tile_pool(name=f"gp{g}", bufs=1, side="left"))

    def loadw(m):
        w = gwp.tile([P, DC, D], BF16, name=f"gwt_{m}", tag="gwt")
        nc.sync.dma_start(out=w, in_=gw_d[(g, m)][:].rearrange("(kc p) n -> p kc n", p=P))
        return w

    wr, ur = loadw("Wr"), loadw("Ur")
    rx = gper.tile([P, DC, 512], BF16, name="rx")
    for n in range(DC):
        ps = PS()
        for k in range(DC):
            nc.tensor.matmul(ps, lhsT=wr[:, k, n * P:(n + 1) * P],
                             rhs=yT[:, k, :], start=(k == 0), stop=False)
        for k in range(DC):
            nc.tensor.matmul(ps, lhsT=ur[:, k, n * P:(n + 1) * P],
                             rhs=xT_b[:, k, :], start=False, stop=(k == DC - 1))
        rr = gtmp.tile([P, 512], F32, name="rr")
        nc.scalar.activation(out=rr, in_=ps, func=Act.Sigmoid)
        nc.vector.tensor_mul(rx[:, n, :], rr, xT_f[:, n, :])
    wz, uz = loadw("Wz"), loadw("Uz")
    zt = gper.tile([P, DC, 512], F32, name="zt")
    for n in range(DC):
        ps = PS()
        for k in range(DC):
            nc.tensor.matmul(ps, lhsT=wz[:, k, n * P:(n + 1) * P],
                             rhs=yT[:, k, :], start=(k == 0), stop=False)
        for k in range(DC):
            nc.tensor.matmul(ps, lhsT=uz[:, k, n * P:(n + 1) * P],
                             rhs=xT_b[:, k, :], start=False, stop=(k == DC - 1))
        nc.scalar.activation(out=zt[:, n, :], in_=ps, func=Act.Sigmoid,
                             bias=nbg_sb[:, n:n + 1])
    wg, ug = loadw("Wg"), loadw("Ug")
    for n in range(DC):
        ps = PS()
        for k in range(DC):
            nc.tensor.matmul(ps, lhsT=wg[:, k, n * P:(n + 1) * P],
                             rhs=yT[:, k, :], start=(k == 0), stop=False)
        for k in range(DC):
            nc.tensor.matmul(ps, lhsT=ug[:, k, n * P:(n + 1) * P],
                             rhs=rx[:, k, :], start=False, stop=(k == DC - 1))
        ht = gtmp.tile([P, 512], F32, name="ht")
        nc.scalar.activation(out=ht, in_=ps, func=Act.Tanh)
        nc.vector.tensor_sub(ht, ht, xT_f[:, n, :])
        nc.vector.tensor_mul(ht, ht, zt[:, n, :])
        nc.vector.tensor_add(oT_f[:, n, :], ht, xT_f[:, n, :])
        if oT_b is not None:
            nc.vector.tensor_copy(oT_b[:, n, :], oT_f[:, n, :])


_NC_CACHE = {}


def _get_nc():
    if "nc" not in _NC_CACHE:
        _NC_CACHE["nc"] = _build()
    return _NC_CACHE["nc"]


def _chunk_t(vec):
    n = vec.shape[0] // P
    return np.ascontiguousarray(vec.reshape(n, P).T.astype(np.float32))


def _prep(inputs):
    f32 = np.float32
    bf = ml_dtypes.bfloat16
    inp = np.asarray(inputs["inputs"], f32)
    mem = np.asarray(inputs["memory"], f32)
    pos = np.asarray(inputs["pos_embedding"], f32)[:, 0, :]

    shared = {
        "posT": np.ascontiguousarray(pos.T).astype(bf),
        "u_t": _chunk_t(np.asarray(inputs["u"], f32).reshape(-1)),
        "v_t": _chunk_t(np.asarray(inputs["v"], f32).reshape(-1)),
        "ln1_g": np.asarray(inputs["ln1_g"], f32),
        "ln1_b": np.asarray(inputs["ln1_b"], f32),
        "ln2_g_t": _chunk_t(np.asarray(inputs["ln2_g"], f32)),
        "ln2_b_t": _chunk_t(np.asarray(inputs["ln2_b"], f32)),
        "bkvK_t": _chunk_t(np.asarray(inputs["bkv"], f32)[0:D]),
        "bkvV_row": np.asarray(inputs["bkv"], f32)[D:2 * D].reshape(1, D).astype(bf),
        "bq_t": _chunk_t(np.asarray(inputs["bq"], f32)),
        "bpos_t": _chunk_t(np.asarray(inputs["bpos"], f32)),
        "bproj_t": _chunk_t(np.asarray(inputs["bproj"], f32)),
        "b1_t": _chunk_t(np.asarray(inputs["mlp_b1"], f32)),
        "b2_t": _chunk_t(np.asarray(inputs["mlp_b2"], f32)),
        "nbg1_t": _chunk_t(-np.asarray(inputs["g1_bg"], f32)),
        "nbg2_t": _chunk_t(-np.asarray(inputs["g2_bg"], f32)),
        "Wkv": np.asarray(inputs["Wkv"], f32).astype(bf),
        "Wq": np.asarray(inputs["Wq"], f32).astype(bf),
        "Wpos": np.asarray(inputs["Wpos"], f32).astype(bf),
        "Wproj": np.asarray(inputs["Wproj"], f32).astype(bf),
        "mlp_W1": np.asarray(inputs["mlp_W1"], f32).astype(bf),
        "mlp_W2": np.asarray(inputs["mlp_W2"], f32).astype(bf),
    }
    for g in (1, 2):
        for m in ("Wr", "Ur", "Wz", "Uz", "Wg", "Ug"):
            shared[f"g{g}_{m}"] = np.asarray(inputs[f"g{g}_{m}"], f32).astype(bf)

    in_maps = []
    for b in range(BS):
        im = dict(shared)
        im["x_full"] = np.ascontiguousarray(
            np.concatenate([mem[:, b, :], inp[:, b, :]], axis=0))
        im["inpT"] = np.ascontiguousarray(inp[:, b, :].T)
        in_maps.append(im)
    return in_maps


def kernel(**inputs):
    nc = _get_nc()
    in_maps = _prep(inputs)
    res = run_bass_kernel_spmd(nc, in_maps, core_ids=list(range(BS)))
    out = np.stack([res.results[b]["out"] for b in range(BS)], axis=1)
    return np.ascontiguousarray(out.astype(np.float32))


if __name__ == "__main__":
    _get_nc()
    print("build+compile OK")


# revision 8
# speedup vs baseline: 149.4967x; 149.4967x over previous
"""GTrXL layer (TransformerXL attention + GRU gating) on 8 TRN2 NeuronCores.

Sharding: pure data-parallel over batch (BS=8 -> 1 batch element per core).
No collectives. Per-core Bass/Tile kernel computes the full layer for its
batch element.

Layout convention on-chip: activations are kept TRANSPOSED [feature, token]
(feature on partitions, 128-chunks) so that weight matrices in natural [K, N]
layout serve as the stationary matmul operand and matmul outputs land
transposed again:  outT[n, t] = sum_k W[k, n] * xT[k, t].

Matmul compute in bf16 (f32 accumulate in PSUM); LN/softmax/GRU elementwise
math in f32.

Relative-shift: pos scores P[i, relk] are written per 128-query-chunk to a
DRAM scratch of row stride 1536 whose tail 512 columns are pre-filled with
-1e30; the shifted read  shifted[i, j] = P[i, 511 + j - i]  is a single
strided DMA (row step 1535), and the pad lands exactly on the masked region
j > i + 512, so masking comes for free.
"""

import sys

if '/opt/trn_rl_repo' not in sys.path:
    sys.path.insert(0, '/opt/trn_rl_repo')

import numpy as np
import ml_dtypes

import concourse.bass as bass
import concourse.tile as tile
from concourse import bacc, mybir
from concourse.bass_utils import run_bass_kernel_spmd
from concourse.masks import make_identity

BF16 = mybir.dt.bfloat16
F32 = mybir.dt.float32

HEAD_NUM, HEAD_DIM = 16, 64
D, HID = 1024, 4096
CUR, PREV, BS = 512, 512, 8
FULL = CUR + PREV
EPS = 1e-5
SCALE = 1.0 / (HEAD_DIM ** 0.5)
P = 128
DC = D // P          # 8 feature chunks
HC = HID // P        # 32 hidden chunks
TCF = FULL // P      # 8 full-token chunks
TCC = CUR // P       # 4 query-token chunks
NEG = -1.0e30

AluOp = mybir.AluOpType
Act = mybir.ActivationFunctionType


def _dram_in(dram, name, shape, dtype):
    return dram.tile(list(shape), dtype, kind="ExternalInput", name=name,
                     uniquify=False)


def _mm_chain(nc, psum, lhsT_tiles, rhs_tiles):
    n = len(lhsT_tiles)
    for i in range(n):
        nc.tensor.matmul(psum, lhsT=lhsT_tiles[i], rhs=rhs_tiles[i],
                         start=(i == 0), stop=(i == n - 1))


def _build():
    nc = bacc.Bacc("TRN2", target_bir_lowering=False)
    with tile.TileContext(nc) as tc:
        _emit(nc, tc)
    nc.compile()
    return nc


def _emit(nc, tc):
    from contextlib import ExitStack

    with ExitStack() as root:
        dram = root.enter_context(tc.tile_pool(name="io", bufs=1, space="DRAM"))

        # ---------------- DRAM I/O ----------------
        x_full = _dram_in(dram, "x_full", (FULL, D), F32)
        inpT_d = _dram_in(dram, "inpT", (D, CUR), F32)
        posT_d = _dram_in(dram, "posT", (D, FULL), BF16)
        u_d = _dram_in(dram, "u_t", (P, DC), F32)
        v_d = _dram_in(dram, "v_t", (P, DC), F32)
        ln1g_d = _dram_in(dram, "ln1_g", (D,), F32)
        ln1b_d = _dram_in(dram, "ln1_b", (D,), F32)
        ln2g_d = _dram_in(dram, "ln2_g_t", (P, DC), F32)
        ln2b_d = _dram_in(dram, "ln2_b_t", (P, DC), F32)
        bkvK_d = _dram_in(dram, "bkvK_t", (P, DC), F32)
        bkvV_d = _dram_in(dram, "bkvV_row", (1, D), BF16)
        bq_d = _dram_in(dram, "bq_t", (P, DC), F32)
        bpos_d = _dram_in(dram, "bpos_t", (P, DC), F32)
        bproj_d = _dram_in(dram, "bproj_t", (P, DC), F32)
        b1_d = _dram_in(dram, "b1_t", (P, HC), F32)
        b2_d = _dram_in(dram, "b2_t", (P, DC), F32)
        nbg1_d = _dram_in(dram, "nbg1_t", (P, DC), F32)
        nbg2_d = _dram_in(dram, "nbg2_t", (P, DC), F32)

        wkv_d = _dram_in(dram, "Wkv", (D, 2 * D), BF16)
        wq_d = _dram_in(dram, "Wq", (D, D), BF16)
        wpos_d = _dram_in(dram, "Wpos", (D, D), BF16)
        wproj_d = _dram_in(dram, "Wproj", (D, D), BF16)
        gw_d = {}
        for g in (1, 2):
            for m in ("Wr", "Ur", "Wz", "Uz", "Wg", "Ug"):
                gw_d[(g, m)] = _dram_in(dram, f"g{g}_{m}", (D, D), BF16)
        w1_d = _dram_in(dram, "mlp_W1", (D, HID), BF16)
        w2_d = _dram_in(dram, "mlp_W2", (HID, D), BF16)

        out_d = dram.tile([CUR, D], F32, kind="ExternalOutput", name="out",
                          uniquify=False)

        n_scr = 8
        scr = [dram.tile([P, 1536], BF16, name=f"scr{s}") for s in range(n_scr)]

        # ---------------- constants ----------------
        const = root.enter_context(tc.tile_pool(name="const", bufs=1))
        ident_f = const.tile([P, P], F32)
        make_identity(nc, ident_f)
        ident_b = const.tile([P, P], BF16)
        make_identity(nc, ident_b)
        ones_row = const.tile([1, P], BF16)
        nc.vector.memset(ones_row, 1.0)
        ones_red = const.tile([P, 1], BF16)
        nc.vector.memset(ones_red, 1.0)
        eps_t = const.tile([P, 1], F32)
        nc.vector.memset(eps_t, EPS)

        def cload(name, dref, shape, dtype=F32):
            t = const.tile(list(shape), dtype, name=name)
            nc.sync.dma_start(out=t, in_=dref[:])
            return t

        u_sb = cload("u_sb", u_d, (P, DC))
        v_sb = cload("v_sb", v_d, (P, DC))
        ln2g_sb = cload("ln2g_sb", ln2g_d, (P, DC))
        ln2b_sb = cload("ln2b_sb", ln2b_d, (P, DC))
        bkvK_sb = cload("bkvK_sb", bkvK_d, (P, DC))
        bkvV_sb = cload("bkvV_sb", bkvV_d, (1, D), BF16)
        bq_sb = cload("bq_sb", bq_d, (P, DC))
        bpos_sb = cload("bpos_sb", bpos_d, (P, DC))
        bproj_sb = cload("bproj_sb", bproj_d, (P, DC))
        b1_sb = cload("b1_sb", b1_d, (P, HC))
        b2_sb = cload("b2_sb", b2_d, (P, DC))
        nbg1_sb = cload("nbg1_sb", nbg1_d, (P, DC))
        nbg2_sb = cload("nbg2_sb", nbg2_d, (P, DC))

        padw = const.tile([P, 512], BF16)
        nc.vector.memset(padw, NEG)
        for s in range(n_scr):
            nc.sync.dma_start(out=scr[s][:, 1024:1536], in_=padw)

        # shared psum pools (4 + 2 + 2 = 8 banks)
        psum = root.enter_context(tc.tile_pool(name="psum", bufs=4, space="PSUM"))
        psum_t = root.enter_context(tc.tile_pool(name="psum_t", bufs=2, space="PSUM"))
        psum_s = root.enter_context(tc.tile_pool(name="psum_s", bufs=2, space="PSUM"))

        def PS():
            return psum.tile([P, 512], F32, name="ps", tag="ps")

        def PT(dtype):
            return psum_t.tile([P, P], dtype, name="pt", tag="pt")

        def SM():
            return psum_s.tile([1, 512], F32, name="sm", tag="sm")

        # lifetime-managed activations (two-sided stack allocator:
        # frees must be LIFO per side, so lifetimes are laid out on
        # left/right stacks to nest properly)
        def mk(name, shape, dtype, side):
            t, fr = tc.tile(list(shape), dtype, name=name, side=side)
            return t, fr

        x1T, fr_x1T = mk("x1T", (P, DC, FULL), BF16, "left")

        # ================= Phase 1: LN1 + transpose =================
        with ExitStack() as ph:
            ln1c = ph.enter_context(tc.tile_pool(name="ln1c", bufs=1, side="left"))
            ln1g_sb = ln1c.tile([P, D], F32)
            nc.sync.dma_start(out=ln1g_sb, in_=bass.AP(
                tensor=ln1g_d.tensor, offset=ln1g_d.offset, ap=[[0, P], [1, D]]))
            ln1b_sb = ln1c.tile([P, D], F32)
            nc.sync.dma_start(out=ln1b_sb, in_=bass.AP(
                tensor=ln1b_d.tensor, offset=ln1b_d.offset, ap=[[0, P], [1, D]]))

            xw = ph.enter_context(tc.tile_pool(name="xw", bufs=3, side="left"))
            st = ph.enter_context(tc.tile_pool(name="st", bufs=3, side="left"))
            x_t = x_full[:].rearrange("(tc p) d -> p tc d", p=P)
            for tcx in range(TCF):
                xt = xw.tile([P, D], F32, name="xt")
                nc.sync.dma_start(out=xt, in_=x_t[:, tcx, :])
                stats = st.tile([P, 2, 6], F32, name="stats")
                nc.vector.bn_stats(out=stats[:, 0, :], in_=xt[:, 0:512])
                nc.vector.bn_stats(out=stats[:, 1, :], in_=xt[:, 512:1024])
                mv = st.tile([P, 2], F32, name="mv")
                nc.vector.bn_aggr(out=mv, in_=stats)
                sd = st.tile([P, 1], F32, name="sd")
                nc.scalar.activation(out=sd, in_=mv[:, 1:2], func=Act.Sqrt,
                                     bias=eps_t)
                rstd = st.tile([P, 1], F32, name="rstd")
                nc.vector.reciprocal(out=rstd, in_=sd)
                xn = xw.tile([P, D], F32, name="xn")
                nc.vector.tensor_scalar(out=xn, in0=xt, scalar1=mv[:, 0:1],
                                        scalar2=rstd, op0=AluOp.subtract,
                                        op1=AluOp.mult)
                x1n = xw.tile([P, D], F32, name="x1n")
                nc.vector.scalar_tensor_tensor(out=x1n, in0=xn, scalar=1.0,
                                               in1=ln1g_sb, op0=AluOp.mult,
                                               op1=AluOp.mult)
                nc.vector.tensor_add(x1n, x1n, ln1b_sb)
                for dc in range(DC):
                    pt = PT(F32)
                    nc.tensor.transpose(pt, x1n[:, dc * P:(dc + 1) * P], ident_f)
                    nc.vector.tensor_copy(x1T[:, dc, tcx * P:(tcx + 1) * P], pt)

        # ================= Phase 2: KT, V, qT, rT =================
        kT, fr_kT = mk("kT", (P, DC, FULL), BF16, "right")
        v_nat, fr_v = mk("v_nat", (P, TCF, D), BF16, "right")
        rT, fr_rT = mk("rT", (P, DC, FULL), BF16, "right")
        quT, fr_quT = mk("quT", (P, DC, CUR), BF16, "right")
        qvT, fr_qvT = mk("qvT", (P, DC, CUR), BF16, "right")

        with ExitStack() as ph:
            wkvp = ph.enter_context(tc.tile_pool(name="wkvp", bufs=1, side="right"))
            wkv = wkvp.tile([P, DC, 2 * D], BF16)
            nc.sync.dma_start(out=wkv, in_=wkv_d[:].rearrange("(kc p) n -> p kc n", p=P))
            for n in range(DC):
                for th in range(2):
                    ps = PS()
                    _mm_chain(nc, ps,
                              [wkv[:, k, n * P:(n + 1) * P] for k in range(DC)],
                              [x1T[:, k, th * 512:(th + 1) * 512] for k in range(DC)])
                    nc.vector.tensor_scalar_add(kT[:, n, th * 512:(th + 1) * 512],
                                                ps, bkvK_sb[:, n:n + 1])
            for t in range(TCF):
                for nh in range(2):
                    ps = PS()
                    for k in range(DC):
                        nc.tensor.matmul(ps, lhsT=x1T[:, k, t * P:(t + 1) * P],
                                         rhs=wkv[:, k, D + nh * 512:D + (nh + 1) * 512],
                                         start=(k == 0), stop=False)
                    nc.tensor.matmul(ps, lhsT=ones_row,
                                     rhs=bkvV_sb[:, nh * 512:(nh + 1) * 512],
                                     start=False, stop=True)
                    nc.vector.tensor_copy(v_nat[:, t, nh * 512:(nh + 1) * 512], ps)
        with ExitStack() as ph:
            wqp = ph.enter_context(tc.tile_pool(name="wqp", bufs=1, side="right"))
            wq = wqp.tile([P, DC, D], BF16)
            nc.sync.dma_start(out=wq, in_=wq_d[:].rearrange("(kc p) n -> p kc n", p=P))
            qw = ph.enter_context(tc.tile_pool(name="qw", bufs=3, side="right"))
            for n in range(DC):
                ps = PS()
                _mm_chain(nc, ps,
                          [wq[:, k, n * P:(n + 1) * P] for k in range(DC)],
                          [x1T[:, k, CUR:FULL] for k in range(DC)])
                qn = qw.tile([P, 512], F32, name="qn")
                nc.vector.tensor_scalar_add(qn, ps, bq_sb[:, n:n + 1])
                nc.vector.tensor_scalar_add(quT[:, n, :], qn, u_sb[:, n:n + 1])
                nc.vector.tensor_scalar_add(qvT[:, n, :], qn, v_sb[:, n:n + 1])
        with ExitStack() as ph:
            wpp = ph.enter_context(tc.tile_pool(name="wpp", bufs=1, side="right"))
            wpos = wpp.tile([P, DC, D], BF16)
            nc.sync.dma_start(out=wpos, in_=wpos_d[:].rearrange("(kc p) n -> p kc n", p=P))
            posT_sb = wpp.tile([P, DC, FULL], BF16)
            nc.sync.dma_start(out=posT_sb, in_=posT_d[:].rearrange("(kc p) f -> p kc f", p=P))
            for n in range(DC):
                for fh in range(2):
                    ps = PS()
                    _mm_chain(nc, ps,
                              [wpos[:, k, n * P:(n + 1) * P] for k in range(DC)],
                              [posT_sb[:, k, fh * 512:(fh + 1) * 512] for k in range(DC)])
                    nc.vector.tensor_scalar_add(rT[:, n, fh * 512:(fh + 1) * 512],
                                                ps, bpos_sb[:, n:n + 1])
        fr_x1T()

        # ================= Phase 3: attention =================
        avT, fr_avT = mk("avT", (P, DC, CUR), BF16, "left")
        with ExitStack() as ph:
            aw = ph.enter_context(tc.tile_pool(name="aw", bufs=3, side="left"))
            atw = ph.enter_context(tc.tile_pool(name="atw", bufs=2, side="left"))
            rw = ph.enter_context(tc.tile_pool(name="rw", bufs=2, side="left"))
            scnt = 0
            for h in range(HEAD_NUM):
                ch, rb = h // 2, (h % 2) * HEAD_DIM
                quh = quT[rb:rb + HEAD_DIM, ch, :]
                qvh = qvT[rb:rb + HEAD_DIM, ch, :]
                kh = kT[rb:rb + HEAD_DIM, ch, :]
                rh = rT[rb:rb + HEAD_DIM, ch, :]
                attnT = atw.tile([P, TCF, 512], BF16, name="attnT")
                shps = []
                for ic in range(TCC):
                    s_t = scr[scnt % n_scr]
                    scnt += 1
                    pb = aw.tile([P, FULL], BF16, name="pb", bufs=4)
                    for jh in range(2):
                        pp = PS()
                        nc.tensor.matmul(pp, lhsT=qvh[:, ic * P:(ic + 1) * P],
                                         rhs=rh[:, jh * 512:(jh + 1) * 512],
                                         start=True, stop=True)
                        nc.scalar.copy(pb[:, jh * 512:(jh + 1) * 512], pp)
                    nc.sync.dma_start(out=s_t[:, 0:1024], in_=pb)
                    shp = aw.tile([P, FULL], BF16, name="shp", bufs=5)
                    shift_ap = bass.AP(tensor=s_t.tensor, offset=s_t.offset + 511,
                                       ap=[[1535, P], [1, FULL]])
                    nc.sync.dma_start(out=shp, in_=shift_ap)
                    shps.append(shp)
                for ic in range(TCC):
                    shp = shps[ic]
                    es = aw.tile([P, FULL], BF16, name="es")
                    for jh in range(2):
                        cp = PS()
                        nc.tensor.matmul(cp, lhsT=quh[:, ic * P:(ic + 1) * P],
                                         rhs=kh[:, jh * 512:(jh + 1) * 512],
                                         start=True, stop=True)
                        sm = aw.tile([P, 512], F32, name="smadd")
                        nc.vector.tensor_add(sm, cp, shp[:, jh * 512:(jh + 1) * 512])
                        nc.scalar.activation(out=es[:, jh * 512:(jh + 1) * 512],
                                             in_=sm, func=Act.Exp, scale=SCALE)
                    for jc in range(TCF):
                        if jc > ic + 4:
                            nc.vector.memset(attnT[:, jc, ic * P:(ic + 1) * P], 0.0)
                            continue
                        pt = PT(BF16)
                        nc.tensor.transpose(pt, es[:, jc * P:(jc + 1) * P], ident_b)
                        if jc % 2 == 0:
                            nc.vector.tensor_copy(attnT[:, jc, ic * P:(ic + 1) * P], pt)
                        else:
                            nc.scalar.copy(attnT[:, jc, ic * P:(ic + 1) * P], pt)
                dn = SM()
                _mm_chain(nc, dn, [ones_red] * TCF,
                          [attnT[:, jc, :] for jc in range(TCF)])
                recip = rw.tile([1, 512], F32, name="recip")
                nc.vector.reciprocal(out=recip, in_=dn)
                recipB = rw.tile([HEAD_DIM, 512], F32, name="recipB")
                nc.gpsimd.partition_broadcast(recipB, recip)
                av = PS()
                for jc in range(TCF):
                    nc.tensor.matmul(av[0:HEAD_DIM, :],
                                     lhsT=v_nat[:, jc, h * HEAD_DIM:(h + 1) * HEAD_DIM],
                                     rhs=attnT[:, jc, :],
                                     start=(jc == 0), stop=(jc == TCF - 1))
                nc.vector.tensor_mul(avT[rb:rb + HEAD_DIM, ch, :],
                                     av[0:HEAD_DIM, :], recipB)
        fr_qvT(); fr_quT(); fr_rT(); fr_v(); fr_kT()

        # ================= Phase 4: proj + GRU1 =================
        a1T, fr_a1T = mk("a1T", (P, DC, CUR), BF16, "right")
        with ExitStack() as ph:
            wpr = ph.enter_context(tc.tile_pool(name="wpr", bufs=1, side="left"))
            wproj = wpr.tile([P, DC, D], BF16)
            nc.sync.dma_start(out=wproj, in_=wproj_d[:].rearrange("(kc p) n -> p kc n", p=P))
            for n in range(DC):
                ps = PS()
                _mm_chain(nc, ps,
                          [wproj[:, k, n * P:(n + 1) * P] for k in range(DC)],
                          [avT[:, k, :] for k in range(DC)])
                nc.vector.tensor_scalar(out=a1T[:, n, :], in0=ps,
                                        scalar1=bproj_sb[:, n:n + 1],
                                        scalar2=0.0, op0=AluOp.add,
                                        op1=AluOp.max)
        fr_avT()

        o1T_f, fr_o1f = mk("o1T_f", (P, DC, CUR), F32, "left")
        o1T_b, fr_o1b = mk("o1T_b", (P, DC, CUR), BF16, "left")
        inpT_f, fr_inpf = mk("inpT_f", (P, DC, CUR), F32, "left")
        inpT_b, fr_inpb = mk("inpT_b", (P, DC, CUR), BF16, "left")
        nc.sync.dma_start(out=inpT_f, in_=inpT_d[:].rearrange("(kc p) t -> p kc t", p=P))
        nc.vector.tensor_copy(inpT_b, inpT_f)
        with ExitStack() as ph:
            _gru(nc, tc, ph, PS, gw_d, 1, a1T, inpT_b, inpT_f, nbg1_sb,
                 o1T_f, o1T_b)
        fr_inpb(); fr_inpf(); fr_a1T()

        # ================= Phase 5: LN2 =================
        x2T, fr_x2T = mk("x2T", (P, DC, CUR), BF16, "right")
        with ExitStack() as ph:
            lw = ph.enter_context(tc.tile_pool(name="lw", bufs=2, side="left"))
            sqp = ph.enter_context(tc.tile_pool(name="sqp", bufs=1, side="left"))
            sq = sqp.tile([P, DC, 512], BF16, name="sq")
            for n in range(DC):
                nc.vector.tensor_mul(sq[:, n, :], o1T_f[:, n, :], o1T_f[:, n, :])
            s1 = SM()
            _mm_chain(nc, s1, [ones_red] * DC, [o1T_b[:, n, :] for n in range(DC)])
            mean = lw.tile([1, 512], F32, name="mean")
            nc.vector.tensor_scalar_mul(mean, s1, 1.0 / D)
            s2 = SM()
            _mm_chain(nc, s2, [ones_red] * DC, [sq[:, n, :] for n in range(DC)])
            m2m = lw.tile([1, 512], F32, name="m2m")
            nc.vector.tensor_scalar_mul(m2m, s2, 1.0 / D)
            var = lw.tile([1, 512], F32, name="var")
            nc.vector.scalar_tensor_tensor(out=var, in0=mean, scalar=1.0,
                                           in1=mean, op0=AluOp.mult,
                                           op1=AluOp.mult)
            nc.vector.tensor_sub(var, m2m, var)
            sd = lw.tile([1, 512], F32, name="sd2")
            nc.scalar.activation(out=sd, in_=var, func=Act.Sqrt,
                                 bias=eps_t[0:1, :])
            rstd = lw.tile([1, 512], F32, name="rstd2")
            nc.vector.reciprocal(out=rstd, in_=sd)
            meanB = lw.tile([P, 512], F32, name="meanB")
            nc.gpsimd.partition_broadcast(meanB, mean)
            rstdB = lw.tile([P, 512], F32, name="rstdB")
            nc.gpsimd.partition_broadcast(rstdB, rstd)
            for n in range(DC):
                t1 = lw.tile([P, 512], F32, name="t1")
                nc.vector.tensor_sub(t1, o1T_f[:, n, :], meanB)
                nc.vector.tensor_mul(t1, t1, rstdB)
                nc.vector.tensor_scalar(out=x2T[:, n, :], in0=t1,
                                        scalar1=ln2g_sb[:, n:n + 1],
                                        scalar2=ln2b_sb[:, n:n + 1],
                                        op0=AluOp.mult, op1=AluOp.add)

        # ================= Phase 6: MLP =================
        with ExitStack() as ph6:
            m1w = ph6.enter_context(tc.tile_pool(name="m1w", bufs=1, side="right"))
            m1T = m1w.tile([P, HC, 512], BF16)
            with ExitStack() as ph:
                w1p = ph.enter_context(tc.tile_pool(name="w1p", bufs=4, side="right"))
                w1r = w1_d[:].rearrange("(kc p) n -> p kc n", p=P)
                for n in range(HC):
                    w1t = w1p.tile([P, DC, P], BF16, name="w1t", tag="w1t")
                    nc.sync.dma_start(out=w1t, in_=w1r[:, :, n * P:(n + 1) * P])
                    ps = PS()
                    _mm_chain(nc, ps,
                              [w1t[:, k, :] for k in range(DC)],
                              [x2T[:, k, :] for k in range(DC)])
                    nc.vector.tensor_scalar(out=m1T[:, n, :], in0=ps,
                                            scalar1=b1_sb[:, n:n + 1],
                                            scalar2=0.0, op0=AluOp.add,
                                            op1=AluOp.max)
            m2T, fr_m2T = mk("m2T", (P, DC, CUR), BF16, "left")
            w2p = ph6.enter_context(tc.tile_pool(name="w2p", bufs=3, side="left"))
            w2r = w2_d[:].rearrange("(kc p) n -> p kc n", p=P)
            for n in range(DC):
                w2t = w2p.tile([P, HC, P], BF16, name="w2t", tag="w2t")
                nc.sync.dma_start(out=w2t, in_=w2r[:, :, n * P:(n + 1) * P])
                ps = PS()
                _mm_chain(nc, ps,
                          [w2t[:, k, :] for k in range(HC)],
                          [m1T[:, k, :] for k in range(HC)])
                nc.vector.tensor_scalar(out=m2T[:, n, :], in0=ps,
                                        scalar1=b2_sb[:, n:n + 1],
                                        scalar2=0.0, op0=AluOp.add,
                                        op1=AluOp.max)
        fr_x2T()

        # ================= Phase 7: GRU2 =================
        o2T_f, fr_o2 = mk("o2T_f", (P, DC, CUR), F32, "right")
        with ExitStack() as ph:
            _gru(nc, tc, ph, PS, gw_d, 2, m2T, o1T_b, o1T_f, nbg2_sb,
                 o2T_f, None)
        fr_m2T(); fr_o1b(); fr_o1f()

        # ================= Phase 8: transpose out =================
        with ExitStack() as ph:
            ow = ph.enter_context(tc.tile_pool(name="ow", bufs=2, side="left"))
            for t in range(TCC):
                on = ow.tile([P, D], F32, name="on")
                for n in range(DC):
                    pt = PT(F32)
                    nc.tensor.transpose(pt, o2T_f[:, n, t * P:(t + 1) * P], ident_f)
                    nc.vector.tensor_copy(on[:, n * P:(n + 1) * P], pt)
                nc.sync.dma_start(out=out_d[t * P:(t + 1) * P, :], in_=on)
        fr_o2()


def _gru(nc, tc, ph, PS, gw_d, g, yT, xT_b, xT_f, nbg_sb, oT_f, oT_b):
    gwp = ph.enter_context(tc.tile_pool(name=f"gw{g}", bufs=3, side="left"))
    gtmp = ph.enter_context(tc.tile_pool(name=f"gt{g}", bufs=2, side="left"))
    gper = ph.enter_context(tc.tile_pool(name=f"gp{g}", bufs=1, side="left"))

    def loadw(m):
        w = gwp.tile([P, DC, D], BF16, name=f"gwt_{m}", tag="gwt")
        nc.sync.dma_start(out=w, in_=gw_d[(g, m)][:].rearrange("(kc p) n -> p kc n", p=P))
        return w

    wr, ur = loadw("Wr"), loadw("Ur")
    rx = gper.tile([P, DC, 512], BF16, name="rx")
    for n in range(DC):
        ps = PS()
        for k in range(DC):
            nc.tensor.matmul(ps, lhsT=wr[:, k, n * P:(n + 1) * P],
                             rhs=yT[:, k, :], start=(k == 0), stop=False)
        for k in range(DC):
            nc.tensor.matmul(ps, lhsT=ur[:, k, n * P:(n + 1) * P],
                             rhs=xT_b[:, k, :], start=False, stop=(k == DC - 1))
        rr = gtmp.tile([P, 512], F32, name="rr")
        nc.scalar.activation(out=rr, in_=ps, func=Act.Sigmoid)
        nc.vector.tensor_mul(rx[:, n, :], rr, xT_f[:, n, :])
    wz, uz = loadw("Wz"), loadw("Uz")
    zt = gper.tile([P, DC, 512], F32, name="zt")
    for n in range(DC):
        ps = PS()
        for k in range(DC):
            nc.tensor.matmul(ps, lhsT=wz[:, k, n * P:(n + 1) * P],
                             rhs=yT[:, k, :], start=(k == 0), stop=False)
        for k in range(DC):
            nc.tensor.matmul(ps, lhsT=uz[:, k, n * P:(n + 1) * P],
                             rhs=xT_b[:, k, :], start=False, stop=(k == DC - 1))
        nc.scalar.activation(out=zt[:, n, :], in_=ps, func=Act.Sigmoid,
                             bias=nbg_sb[:, n:n + 1])
    wg, ug = loadw("Wg"), loadw("Ug")
    for n in range(DC):
        ps = PS()
        for k in range(DC):
            nc.tensor.matmul(ps, lhsT=wg[:, k, n * P:(n + 1) * P],
                             rhs=yT[:, k, :], start=(k == 0), stop=False)
        for k in range(DC):
            nc.tensor.matmul(ps, lhsT=ug[:, k, n * P:(n + 1) * P],
                             rhs=rx[:, k, :], start=False, stop=(k == DC - 1))
        ht = gtmp.tile([P, 512], F32, name="ht")
        nc.scalar.activation(out=ht, in_=ps, func=Act.Tanh)
        nc.vector.tensor_sub(ht, ht, xT_f[:, n, :])
        nc.vector.tensor_mul(ht, ht, zt[:, n, :])
        nc.vector.tensor_add(oT_f[:, n, :], ht, xT_f[:, n, :])
        if oT_b is not None:
            nc.vector.tensor_copy(oT_b[:, n, :], oT_f[:, n, :])


_NC_CACHE = {}


def _get_nc():
    if "nc" not in _NC_CACHE:
        _NC_CACHE["nc"] = _build()
    return _NC_CACHE["nc"]


def _chunk_t(vec):
    n = vec.shape[0] // P
    return np.ascontiguousarray(vec.reshape(n, P).T.astype(np.float32))


def _prep(inputs):
    f32 = np.float32
    bf = ml_dtypes.bfloat16
    inp = np.asarray(inputs["inputs"], f32)
    mem = np.asarray(inputs["memory"], f32)
    pos = np.asarray(inputs["pos_embedding"], f32)[:, 0, :]

    shared = {
        "posT": np.ascontiguousarray(pos.T).astype(bf),
        "u_t": _chunk_t(np.asarray(inputs["u"], f32).reshape(-1)),
        "v_t": _chunk_t(np.asarray(inputs["v"], f32).reshape(-1)),
        "ln1_g": np.asarray(inputs["ln1_g"], f32),
        "ln1_b": np.asarray(inputs["ln1_b"], f32),
        "ln2_g_t": _chunk_t(np.asarray(inputs["ln2_g"], f32)),
        "ln2_b_t": _chunk_t(np.asarray(inputs["ln2_b"], f32)),
        "bkvK_t": _chunk_t(np.asarray(inputs["bkv"], f32)[0:D]),
        "bkvV_row": np.asarray(inputs["bkv"], f32)[D:2 * D].reshape(1, D).astype(bf),
        "bq_t": _chunk_t(np.asarray(inputs["bq"], f32)),
        "bpos_t": _chunk_t(np.asarray(inputs["bpos"], f32)),
        "bproj_t": _chunk_t(np.asarray(inputs["bproj"], f32)),
        "b1_t": _chunk_t(np.asarray(inputs["mlp_b1"], f32)),
        "b2_t": _chunk_t(np.asarray(inputs["mlp_b2"], f32)),
        "nbg1_t": _chunk_t(-np.asarray(inputs["g1_bg"], f32)),
        "nbg2_t": _chunk_t(-np.asarray(inputs["g2_bg"], f32)),
        "Wkv": np.asarray(inputs["Wkv"], f32).astype(bf),
        "Wq": np.asarray(inputs["Wq"], f32).astype(bf),
        "Wpos": np.asarray(inputs["Wpos"], f32).astype(bf),
        "Wproj": np.asarray(inputs["Wproj"], f32).astype(bf),
        "mlp_W1": np.asarray(inputs["mlp_W1"], f32).astype(bf),
        "mlp_W2": np.asarray(inputs["mlp_W2"], f32).astype(bf),
    }
    for g in (1, 2):
        for m in ("Wr", "Ur", "Wz", "Uz", "Wg", "Ug"):
            shared[f"g{g}_{m}"] = np.asarray(inputs[f"g{g}_{m}"], f32).astype(bf)

    in_maps = []
    for b in range(BS):
        im = dict(shared)
        im["x_full"] = np.ascontiguousarray(
            np.concatenate([mem[:, b, :], inp[:, b, :]], axis=0))
        im["inpT"] = np.ascontiguousarray(inp[:, b, :].T)
        in_maps.append(im)
    return in_maps


def kernel(**inputs):
    nc = _get_nc()
    in_maps = _prep(inputs)
    res = run_bass_kernel_spmd(nc, in_maps, core_ids=list(range(BS)))
    out = np.stack([res.results[b]["out"] for b in range(BS)], axis=1)
    return np.ascontiguousarray(out.astype(np.float32))


if __name__ == "__main__":
    _get_nc()
    print("build+compile OK")
